# revision 26
# baseline (speedup 1.0000x reference)
"""Trainium2 Bass kernel for nn_BAKTTime: causal-conv frequency layer + LN + causal MHA.

Sharding: pure data-parallel over batch - 8 of the 64 batch items per NeuronCore,
no collectives. Each core runs the same 5-stage software-pipelined program over
its 8 batch items (S=512, D=512, H=8, DK=64).

Numerics (fp8 e4m3 DoubleRow where the error budget allows):
  - conv: 3-term error-compensated split (xh*Wh + xl*Wh + xh*Wl, one PSUM
    group; W pre-scaled x16 so the lo parts stay in fp8 normal range while the
    big diagonal taps stay under e4m3 max 448; LN is scale-invariant so the
    x16 never needs undoing). 24576 -> 18432 PE cyc/batch.
  - q,k projections: single fp8 DoubleRow (h and Wq quantization errors are
    damped through softmax because score magnitudes are ~0.2); the x64*x64
    weight scale rides into the exp() activation scale (0.125/4096).
    16384 -> 4096 PE cyc/batch.
  - v / scores / ctx / out-projection stay bf16 (their quantization error
    would hit the output un-damped; DoubleRow needs fp8).

Scheduling: the PE executes its stream in order, so the per-(head,ki) chain
scores(PE) -> exp(ACT) -> ctx(PE) would idle the PE for ~1us per slot. Three
countermeasures:
  - the causal mask is applied by ADDING a -1e9 triangle to the scores PSUM
    via an extra PE matmul (lhsT=I128, rhs=tri constant) in the same
    accumulation group - exp then produces exact zeros in the masked region
    and the old DVE trim multiply (a cross-engine hop on the critical path)
    disappears;
  - scores for slot i+1 are emitted before ctx for slot i (per-head
    [128,S] score tiles, double-buffered PSUM);
  - conv matmuls of batch b+1 and out-projection matmuls of batch b-3 are
    chopped into small chunks and drained as PE filler inside the attention
    slots, with the LN/bn/h/transpose chain of b+1 emitted per-s-tile along
    the way.

Per-batch dataflow (as in the baseline): conv -> LN -> hT via 16 xbar
transposes -> fp8 cast (Pool) -> projections (v with a ones column per head
so the ctx matmul also yields the softmax denominator row) -> per-head-pair
attention -> denominator gather (Pool SWDGE) -> reciprocal (DVE) -> [1->64]
HWDGE broadcasts -> one big normalize multiply -> output projection ->
one [128, 4, 512] DRAM store. zero_pad (first query row zeroed) comes from a
column-0 -1e9 variant of the triangle for ki=0 plus a +1e-10 denominator
guard.
"""

import sys

sys.path.insert(0, "/opt/trn_rl_repo")

import numpy as np
import ml_dtypes
from contextlib import ExitStack

import concourse.bass as bass
from concourse import bacc
import concourse.mybir as mybir
import concourse.tile as tile
from concourse.bass_utils import run_bass_kernel_spmd

# Force Exp and Ln to resolve to the single table set that contains both
# (natural_log_exp_and_others), so ACT doesn't thrash table loads.
import concourse.hw_specs as _hw_specs

_orig_get_tables = _hw_specs.get_activation_tables


def _patched_get_tables(arch):
    t = dict(_orig_get_tables(arch))
    exp = mybir.ActivationFunctionType.Exp
    ln = mybir.ActivationFunctionType.Ln
    for name, funcs in t.items():
        if name != "natural_log_exp_and_others" and (exp in funcs or ln in funcs):
            t[name] = funcs - {exp, ln}
    return t


_hw_specs.get_activation_tables = _patched_get_tables
bacc.get_activation_tables = _patched_get_tables

B, S, D, H, KW = 64, 512, 512, 8, 3
DK = D // H  # 64
NCORES = 8
BL = B // NCORES  # 8 batches per core
P = 128
NST = S // P  # 4 s-tiles
NIC = D // P  # 4 input-chunks
NG = 2  # channel pair-groups for DoubleRow (256 ch each)
SP = S + 16  # padded x free dim (2 zero cols + 512 + pad; DR pair stride must be 16B-aligned)
EPS = 1e-12
F32 = mybir.dt.float32
BF16 = mybir.dt.bfloat16
FP8 = mybir.dt.float8e4
AF = mybir.ActivationFunctionType
DR = mybir.MatmulPerfMode.DoubleRow
EXP_SCALE = 0.125 / 4096.0  # 1/8 score scale folded with the x64*x64 qk weight scale
NEG = -1.0e9


def build_nc():
    nc = bacc.Bacc("TRN2", target_bir_lowering=False)
    # x: [b][p][(hilo*2+g)*NG*SP + j*SP + s+2] = fp8 part of x[b, s, g*256+j*128+p]
    xt = nc.declare_dram_parameter("xt", [BL, P, 4 * NG * SP], FP8, isOutput=False)
    # wconv: [hilo*2+g][p][j][k][d] = fp8 part of 16*W'[d, g*256+j*128+p, k]
    wconv = nc.declare_dram_parameter("wconv", [4, P, NG, KW, D], FP8, isOutput=False)
    # wq/wk: [g][p][j][d] = fp8(64*Wq[d, g*256+j*128+p])
    wq = nc.declare_dram_parameter("wq", [NG, P, NG, D], FP8, isOutput=False)
    wk = nc.declare_dram_parameter("wk", [NG, P, NG, D], FP8, isOutput=False)
    wv = nc.declare_dram_parameter("wv", [NIC, P, D], BF16, isOutput=False)
    wo = nc.declare_dram_parameter("wo", [NIC, P, D], BF16, isOutput=False)
    # trin[:, 0, :]: -1e9 strictly-below-diagonal triangle; [:, 1, :]: same + col 0
    trin = nc.declare_dram_parameter("trin", [P, 2, P], BF16, isOutput=False)
    ident = nc.declare_dram_parameter("ident", [P, P], BF16, isOutput=False)
    out = nc.declare_dram_parameter("out", [BL, P, NST, D], F32, isOutput=True)

    with ExitStack() as ctx:
        tc = ctx.enter_context(tile.TileContext(nc))
        singles = ctx.enter_context(tc.tile_pool(name="singles", bufs=1))
        xt_pool = ctx.enter_context(tc.tile_pool(name="xt", bufs=2))
        a_pool = ctx.enter_context(tc.tile_pool(name="a", bufs=5))
        stat_pool = ctx.enter_context(tc.tile_pool(name="stat", bufs=4))
        h_pool = ctx.enter_context(tc.tile_pool(name="h", bufs=4))
        ht_pool = ctx.enter_context(tc.tile_pool(name="ht", bufs=2))
        hh_pool = ctx.enter_context(tc.tile_pool(name="hh", bufs=2))
        qk_pool = ctx.enter_context(tc.tile_pool(name="qk", bufs=12))
        v_pool = ctx.enter_context(tc.tile_pool(name="v", bufs=8))
        pt_pool = ctx.enter_context(tc.tile_pool(name="pt", bufs=4))
        dn_pool = ctx.enter_context(tc.tile_pool(name="dn", bufs=2))
        r_pool = ctx.enter_context(tc.tile_pool(name="r", bufs=3))
        cx_pool = ctx.enter_context(tc.tile_pool(name="cx", bufs=5))
        o_pool = ctx.enter_context(tc.tile_pool(name="o", bufs=2))
        ps_a = ctx.enter_context(tc.tile_pool(name="ps_a", bufs=2, space="PSUM"))
        ps_mm = ctx.enter_context(tc.tile_pool(name="ps_mm", bufs=2, space="PSUM"))
        ps_sc = ctx.enter_context(tc.tile_pool(name="ps_sc", bufs=2, space="PSUM"))
        ps_cx = ctx.enter_context(tc.tile_pool(name="ps_cx", bufs=2, space="PSUM"))

        # --- load weights once ---
        wconv_sb = [singles.tile([P, NG, KW, D], FP8, name=f"wconv{t}", tag=f"wconv{t}") for t in range(4)]
        wq_sb = [singles.tile([P, NG, D], FP8, name=f"wq{g}", tag=f"wq{g}") for g in range(NG)]
        wk_sb = [singles.tile([P, NG, D], FP8, name=f"wk{g}", tag=f"wk{g}") for g in range(NG)]
        wv_sb = [singles.tile([P, D], BF16, name=f"wv{i}", tag=f"wv{i}") for i in range(NIC)]
        wo_sb = [singles.tile([P, D], BF16, name=f"wo{i}", tag=f"wo{i}") for i in range(NIC)]
        trin_sb = singles.tile([P, 2, P], BF16, name="trin", tag="trin")
        ident_sb = singles.tile([P, P], BF16, name="ident", tag="ident")
        eps_sb = singles.tile([P, 1], F32, name="eps", tag="eps")
        nc.vector.memset(eps_sb, EPS)
        tiny_sb = singles.tile([P, 1], F32, name="tiny", tag="tiny")
        nc.vector.memset(tiny_sb, 1e-10)
        zero_sb = singles.tile([P, 1], F32, name="zero", tag="zero")
        nc.vector.memset(zero_sb, 0.0)
        for t in range(4):
            nc.gpsimd.dma_start(out=wconv_sb[t], in_=wconv[t])
        nc.gpsimd.dma_start(out=trin_sb, in_=trin[:])
        nc.gpsimd.dma_start(out=ident_sb, in_=ident[:])
        for g in range(NG):
            nc.gpsimd.dma_start(out=wq_sb[g], in_=wq[g])
            nc.gpsimd.dma_start(out=wk_sb[g], in_=wk[g])
        for i in range(NIC):
            nc.gpsimd.dma_start(out=wv_sb[i], in_=wv[i])
        for i in range(NIC):
            nc.gpsimd.dma_start(out=wo_sb[i], in_=wo[i])

        def load_xt(b):
            xsb = xt_pool.tile([P, 4, NG, SP], FP8, name="xsb", tag="xsb")
            nc.sync.dma_start(out=xsb.rearrange("p a j s -> p (a j s)"), in_=xt[b])
            return xsb

        CONV_TERMS = [(0, 0), (1, 0), (0, 1)]  # (x part, w part): hh, lh, hl
        CHUNK = 6  # conv matmuls per filler chunk

        def make_front_filler(b, xsb, result):
            """Filler closures for conv+LN+h+transposes of batch b.

            result: dict that will receive 'ht' and is later finished with the
            hh cast by finish_front."""
            ht_sb = [ht_pool.tile([P, NG, S], BF16, name=f"ht{g}", tag=f"ht{g}") for g in range(NG)]
            hh_sb = [hh_pool.tile([P, NG, S], FP8, name=f"hh{g}", tag=f"hh{g}") for g in range(NG)]
            mv = stat_pool.tile([P, NST, 2], F32, name="mv", tag="mv")
            result["ht"] = ht_sb
            result["hh"] = hh_sb
            chunks = []
            state = {}

            def conv_chunk(st, lo, hi):
                def emit():
                    if lo == 0:
                        state[st] = ps_a.tile([P, D], F32, name="aps", tag="aps")
                    aps = state[st]
                    for n in range(lo, hi):
                        ds, rem = divmod(n, 18)
                        ti, rem2 = divmod(rem, NG * KW)
                        g, k = divmod(rem2, KW)
                        xi, wi = CONV_TERMS[ti]
                        nc.tensor.matmul(
                            aps[:, ds * 256 : (ds + 1) * 256],
                            lhsT=xsb[:, xi * NG + g, :, st * P + k : st * P + k + P],
                            rhs=wconv_sb[wi * NG + g][:, :, k, ds * 256 : (ds + 1) * 256],
                            start=(rem == 0),
                            stop=(rem == 17),
                            perf_mode=DR,
                        )
                    if hi == 36:
                        # LN + h + transposes for this s-tile
                        asb = a_pool.tile([P, D], F32, name="asb", tag="asb")
                        nc.vector.tensor_copy(asb, aps)
                        stats = stat_pool.tile([P, 6], F32, name="bnst", tag="bnst")
                        nc.vector.bn_stats(out=stats, in_=asb)
                        nc.vector.bn_aggr(out=mv[:, st, :], in_=stats)
                        lnv = stat_pool.tile([P, 1], F32, name="lnv", tag="lnv")
                        rstd = stat_pool.tile([P, 1], F32, name="rstd", tag="rstd")
                        nc.scalar.activation(lnv, mv[:, st, 1:2], AF.Ln, bias=eps_sb, scale=1.0)
                        nc.scalar.activation(rstd, lnv, AF.Exp, bias=zero_sb, scale=-0.5)
                        hsb = h_pool.tile([P, D], BF16, name="hsb", tag="hsb")
                        nc.vector.tensor_scalar(
                            hsb,
                            asb,
                            scalar1=mv[:, st, 0:1],
                            scalar2=rstd,
                            op0=mybir.AluOpType.subtract,
                            op1=mybir.AluOpType.mult,
                        )
                        for i in range(NIC):
                            nc.sync.dma_start(
                                out=ht_sb[i // 2][:, i % 2, st * P : (st + 1) * P],
                                in_=hsb[:, i * P : (i + 1) * P],
                                transpose=True,
                            )

                return emit

            for st in range(NST):
                for c in range(0, 36, CHUNK):
                    chunks.append(conv_chunk(st, c, min(c + CHUNK, 36)))
            return chunks

        def make_outproj_filler(b, csbn):
            osb = o_pool.tile([P, NST, D], F32, name="osb", tag="osb")
            chunks = []

            def op_chunk(st):
                def emit():
                    ops = ps_mm.tile([P, D], F32, name="qps", tag="qps")
                    for hp in range(H // 2):
                        nc.tensor.matmul(
                            ops,
                            lhsT=csbn[:, hp, st * P : (st + 1) * P],
                            rhs=wo_sb[hp],
                            start=(hp == 0),
                            stop=(hp == H // 2 - 1),
                        )
                    nc.scalar.copy(osb[:, st, :], ops)
                    if st == NST - 1:
                        nc.sync.dma_start(out=out[b], in_=osb)

                return emit

            for st in range(NST):
                chunks.append(op_chunk(st))
            return chunks

        def emit_qkv(b, ht_sb, hh_sb):
            v_aug = []
            for st in range(NST):
                vps = ps_mm.tile([P, D], F32, name="qps", tag="qps")
                for i in range(NIC):
                    nc.tensor.matmul(
                        vps,
                        lhsT=ht_sb[i // 2][:, i % 2, st * P : (st + 1) * P],
                        rhs=wv_sb[i],
                        start=(i == 0),
                        stop=(i == NIC - 1),
                    )
                vsb = v_pool.tile([P, H, 66], BF16, name="vsb", tag="vsb")
                nc.vector.memset(vsb[:, :, 64:66], 1.0)
                nc.vector.tensor_copy(
                    vsb[:, :, 0:64], vps.rearrange("p (h d) -> p h d", h=H)
                )
                v_aug.append(vsb)

            qt_sb = []
            kt_sb = []
            for oc in range(NIC):
                for dst, w_sb in ((qt_sb, wq_sb), (kt_sb, wk_sb)):
                    qps = ps_mm.tile([P, S], F32, name="qps", tag="qps")
                    for ss in range(2):
                        for g in range(NG):
                            nc.tensor.matmul(
                                qps[:, ss * 256 : (ss + 1) * 256],
                                lhsT=w_sb[g][:, :, oc * P : (oc + 1) * P],
                                rhs=hh_sb[g][:, :, ss * 256 : ss * 256 + 256],
                                start=(g == 0),
                                stop=(g == NG - 1),
                                perf_mode=DR,
                            )
                    qsb = qk_pool.tile([P, S], BF16, name="qtsb", tag="qtsb")
                    nc.vector.tensor_copy(qsb, qps)
                    dst.append(qsb)
            return qt_sb, kt_sb, v_aug

        def emit_attention(b, qt_sb, kt_sb, v_aug, filler):
            """Per-(pair, ki, head) attention with scores one slot ahead of ctx
            and filler chunks drained into the gaps."""
            dtmp = dn_pool.tile([65, H, S], BF16, name="dtmp", tag="dtmp", bufs=1)
            csb = cx_pool.tile([P, H // 2, S], BF16, name="csb", tag="csb")
            codd = cx_pool.tile([DK, H // 2, S], BF16, name="codd", tag="codd", bufs=2)

            def draw():
                if filler:
                    filler.pop(0)()

            for hp in range(H // 2):
                cps2 = [
                    ps_cx.tile([65, S], F32, name="cps", tag="cps") for _ in range(2)
                ]
                pend = None  # (ki, e, pt)
                for ki in range(NST):
                    qoff = ki * P
                    nq = S - qoff
                    for e in range(2):
                        hr = e * DK
                        sps = ps_sc.tile([P, S], F32, name="sps", tag="sps")
                        # scores first (start=True zeroes the full [0:nq]),
                        # then the -1e9 causal triangle accumulates onto the
                        # 128-wide diagonal block.
                        nc.tensor.matmul(
                            sps[:, 0:nq],
                            lhsT=kt_sb[hp][hr : hr + DK, ki * P : (ki + 1) * P],
                            rhs=qt_sb[hp][hr : hr + DK, qoff:S],
                            start=True,
                            stop=False,
                        )
                        nc.tensor.matmul(
                            sps[:, 0:P],
                            lhsT=ident_sb,
                            rhs=trin_sb[:, 1 if ki == 0 else 0, :],
                            start=False,
                            stop=True,
                        )
                        pt = pt_pool.tile([P, S], BF16, name="pt", tag="pt")
                        nc.scalar.activation(pt[:, 0:nq], sps[:, 0:nq], AF.Exp, scale=EXP_SCALE)
                        if pend is not None:
                            pki, pe, ppt = pend
                            pnq = S - pki * P
                            nc.tensor.matmul(
                                cps2[pe][:, pki * P : S],
                                lhsT=v_aug[pki][:, 2 * hp + pe, 0:65],
                                rhs=ppt[:, 0:pnq],
                                start=(pki == 0),
                                stop=(pki == NST - 1),
                            )
                        draw()
                        pend = (ki, e, pt)
                pki, pe, ppt = pend
                nc.tensor.matmul(
                    cps2[pe][:, pki * P : S],
                    lhsT=v_aug[pki][:, 2 * hp + pe, 0:65],
                    rhs=ppt[:, 0 : S - pki * P],
                    start=(pki == 0),
                    stop=(pki == NST - 1),
                )
                # denominator rows -> staging (row 64), +tiny guard for q=0
                nc.vector.tensor_scalar(
                    dtmp[64:65, 2 * hp, :], cps2[0][64:65, :],
                    scalar1=tiny_sb[64:65, :], scalar2=None, op0=mybir.AluOpType.add,
                )
                nc.vector.tensor_scalar(
                    dtmp[64:65, 2 * hp + 1, :], cps2[1][64:65, :],
                    scalar1=tiny_sb[64:65, :], scalar2=None, op0=mybir.AluOpType.add,
                )
                # unnormalized ctx: even head -> rows 0-63 direct; odd head
                # staged for one partition-shifting DMA at the end.
                nc.vector.tensor_copy(csb[0:DK, hp, :], cps2[0][0:DK, :])
                nc.vector.tensor_copy(codd[:, hp, :], cps2[1][0:DK, :])

            nc.sync.dma_start(out=csb[DK:P, :, :], in_=codd)
            dcat = dn_pool.tile([H, S], F32, name="dcat", tag="dcat")
            nc.gpsimd.dma_start(out=dcat, in_=dtmp[64:65, :, :])
            return (b, csb, dcat)

        def denom_chain(b, dcat):
            rcat = dn_pool.tile([H, S], F32, name="rcat", tag="rcat")
            nc.vector.reciprocal_approx_fast(out=rcat, in_=dcat)
            rcb = dn_pool.tile([H, S], BF16, name="rcb", tag="rcb")
            nc.vector.tensor_copy(rcb, rcat)
            rec = r_pool.tile([P, H // 2, S], BF16, name="rec", tag="rec")
            for hp in range(H // 2):
                # one DMA per pair: src iterates (row in {2hp, 2hp+1}, dup 64, s)
                # which matches dest partitions p = 64*row + dup exactly.
                rsrc = rcb[2 * hp : 2 * hp + 2, :]
                rsrc = bass.AP(
                    tensor=rsrc.tensor,
                    offset=rsrc.offset,
                    ap=[rsrc.ap[0], [0, DK], [1, S]],
                )
                nc.sync.dma_start(out=rec[:, hp, :], in_=rsrc)
            return rec

        def tail_norm(b, csb, rec):
            csbn = cx_pool.tile([P, H // 2, S], BF16, name="csbn", tag="csbn", bufs=3)
            nc.vector.tensor_mul(csbn, csb, rec)
            return (b, csbn)

        # ---------------- pipeline ----------------
        # state per stage, keyed by batch:
        #   front(b):   conv filler emitted during iteration b; ht/hh ready end of b
        #   mid(b):     qkv + attention emitted in iteration b+1
        #   denom(b):   reciprocal+broadcasts in iteration b+2
        #   norm(b):    csbn multiply in iteration b+3
        #   outproj(b): filler during iteration b+4
        front_res = {}  # b -> {"ht":..., "hh":...}
        mid_res = {}  # b -> (b, csb, dcat)
        rec_res = {}  # b -> rec
        norm_res = {}  # b -> (b, csbn)

        xsb_cur = load_xt(0)
        for b in range(BL + 4):
            filler = []
            if b < BL:
                if b + 1 < BL:
                    xsb_next = load_xt(b + 1)
                front_res[b] = {}
                filler += make_front_filler(b, xsb_cur, front_res[b])
                if b + 1 < BL:
                    xsb_cur = xsb_next
            if b - 3 >= 0 and (b - 3) in norm_res:
                nb, csbn = norm_res.pop(b - 3)
                filler += make_outproj_filler(nb, csbn)
            if b - 2 >= 0 and (b - 2) in rec_res:
                mb, csb_, dcat_ = mid_res.pop(b - 2)
                norm_res[b - 2] = tail_norm(mb, csb_, rec_res.pop(b - 2))
            if b - 1 >= 0 and (b - 1) in front_res:
                # pre-draw a few ready conv chunks so the PE has work while
                # the hh cast of b-1 settles
                for _ in range(min(3, len(filler))):
                    filler.pop(0)()
                fr = front_res.pop(b - 1)
                qt, kt, va = emit_qkv(b - 1, fr["ht"], fr["hh"])
                mid_res[b - 1] = emit_attention(b - 1, qt, kt, va, filler)
            # drain leftover filler (startup/drain iterations)
            for f in filler:
                f()
            if b < BL:
                fr = front_res[b]
                for g in range(NG):
                    nc.gpsimd.tensor_copy(fr["hh"][g], fr["ht"][g])
            if b - 1 >= 0 and (b - 1) in mid_res:
                _, _, dcat_ = mid_res[b - 1]
                rec_res[b - 1] = denom_chain(b - 1, dcat_)

    nc.compile()
    return nc


def prep_inputs(inputs):
    """Host-side prep: shard over batch, fold scales into weights, fp8 splits."""
    x = np.asarray(inputs["x"], np.float32)
    conv_w = np.asarray(inputs["conv_w"], np.float32)
    conv_b = np.asarray(inputs["conv_b"], np.float32)
    sb = np.asarray(inputs["sqrt_beta"], np.float32).reshape(D)
    ln_w = np.asarray(inputs["ln_w"], np.float32)
    ln_b = np.asarray(inputs["ln_b"], np.float32)
    Wq = np.asarray(inputs["Wq"], np.float32)
    Wk = np.asarray(inputs["Wk"], np.float32)
    Wv = np.asarray(inputs["Wv"], np.float32)
    Wo = np.asarray(inputs["Wo"], np.float32)
    mask = np.asarray(inputs["mask"])

    for nm in ("bq", "bk", "bv", "bo"):
        assert not np.any(np.asarray(inputs[nm])), f"{nm} must be zero"
    assert not np.any(conv_b), "conv_b must be zero"
    assert not np.any(ln_b), "ln_b must be zero"
    assert np.array_equal(
        mask.reshape(S, S), np.tril(np.ones((S, S), mask.dtype))
    ), "mask must be causal"

    bf = ml_dtypes.bfloat16
    f8 = ml_dtypes.float8_e4m3fn

    c1 = 1.0 - sb * sb
    c2 = 1.0 + sb * sb
    Wp = conv_w * c1[:, None, None]  # [o, i, k]
    Wp[np.arange(D), np.arange(D), 2] += c2
    Wp16 = Wp * 16.0  # x16: diag taps (up to ~17) must stay under e4m3 max 448
    Wph = Wp16.astype(f8)
    Wpl = (Wp16 - Wph.astype(np.float32)).astype(f8)
    # wconv[hilo*2+g][p][j][k][d] = part[d, g*256+j*128+p, k]
    wconv = np.empty((4, P, NG, KW, D), f8)
    for t, Wpart in enumerate((Wph, Wpl)):
        r = np.ascontiguousarray(Wpart.transpose(1, 2, 0)).reshape(NG, NG, P, KW, D)
        wconv[2 * t : 2 * t + 2] = r.transpose(0, 2, 1, 3, 4)

    def fold_qk(W):  # [o, i] -> [g, p, j, o], fp8(64*W*ln_w)
        Wf = (64.0 * W * ln_w[None, :]).astype(f8)
        r = np.ascontiguousarray(Wf.T).reshape(NG, NG, P, D)
        return r.transpose(0, 2, 1, 3).copy()

    wq_h, wk_h = fold_qk(Wq), fold_qk(Wk)

    def fold(W):  # [o, i] -> [ic, il, o] with ln_w folded on i
        Wf = W * ln_w[None, :]
        return np.ascontiguousarray(Wf.T).reshape(NIC, P, D)

    wv_h = fold(Wv).astype(bf)
    wo_h = np.ascontiguousarray(Wo.T).reshape(NIC, P, D).astype(bf)

    # trin[k, v, q]: v=0 plain causal (-1e9 where q < k), v=1 also col 0 = -1e9
    tri = np.where(np.arange(P)[None, :] < np.arange(P)[:, None], NEG, 0.0).astype(np.float32)
    tri0 = tri.copy()
    tri0[:, 0] = NEG
    trin = np.stack([tri, tri0], axis=1)  # [P, 2, P]

    consts = {
        "wconv": wconv,
        "wq": wq_h,
        "wk": wk_h,
        "wv": wv_h,
        "wo": wo_h,
        "trin": trin.astype(bf),
        "ident": np.eye(P, dtype=bf),
    }

    in_maps = []
    for c in range(NCORES):
        xs = x[c * BL : (c + 1) * BL]  # [BL, S, D]
        xh = xs.astype(f8)
        xl = (xs - xh.astype(np.float32)).astype(f8)
        xtp = np.zeros((BL, P, 4, NG, SP), f8)
        for t, xpart in enumerate((xh, xl)):
            r = np.ascontiguousarray(xpart.transpose(0, 2, 1)).reshape(BL, NG, NG, P, S)
            xtp[:, :, 2 * t : 2 * t + 2, :, 2 : 2 + S] = r.transpose(0, 3, 1, 2, 4)
        m = dict(consts)
        m["xt"] = np.ascontiguousarray(xtp).reshape(BL, P, 4 * NG * SP)
        in_maps.append(m)
    return in_maps


_NC_CACHE = {}


def get_nc():
    if "nc" not in _NC_CACHE:
        _NC_CACHE["nc"] = build_nc()
    return _NC_CACHE["nc"]


def unpack_out(arr):
    # [BL, P, NST, D] -> [BL, S, D] (s = st*P + p)
    a = np.asarray(arr, np.float32).reshape(BL, P, NST, D)
    return np.ascontiguousarray(a.transpose(0, 2, 1, 3)).reshape(BL, S, D)


def kernel(**inputs):
    nc = get_nc()
    in_maps = prep_inputs(inputs)
    res = run_bass_kernel_spmd(nc, in_maps, list(range(NCORES)))
    return np.concatenate([unpack_out(r["out"]) for r in res.results], axis=0)


if __name__ == "__main__":
    nc = build_nc()
    print("built ok")


# revision 27
# speedup vs baseline: 1.0281x; 1.0281x over previous
"""Trainium2 Bass kernel for nn_BAKTTime: causal-conv frequency layer + LN + causal MHA.

Sharding: pure data-parallel over batch - 8 of the 64 batch items per NeuronCore,
no collectives. Each core runs the same 5-stage software-pipelined program over
its 8 batch items (S=512, D=512, H=8, DK=64).

Numerics (fp8 e4m3 DoubleRow where the error budget allows):
  - conv: 3-term error-compensated split (xh*Wh + xl*Wh + xh*Wl, one PSUM
    group; W pre-scaled x16 so the lo parts stay in fp8 normal range while the
    big diagonal taps stay under e4m3 max 448; LN is scale-invariant so the
    x16 never needs undoing). 24576 -> 18432 PE cyc/batch.
  - q,k projections: single fp8 DoubleRow (h and Wq quantization errors are
    damped through softmax because score magnitudes are ~0.2); the x64*x64
    weight scale rides into the exp() activation scale (0.125/4096).
    16384 -> 4096 PE cyc/batch.
  - v / scores / ctx / out-projection stay bf16 (their quantization error
    would hit the output un-damped; DoubleRow needs fp8).

Scheduling: the PE executes its stream in order, so the per-(head,ki) chain
scores(PE) -> exp(ACT) -> ctx(PE) would idle the PE for ~1us per slot. Three
countermeasures:
  - the causal mask is applied by ADDING a -1e9 triangle to the scores PSUM
    via an extra PE matmul (lhsT=I128, rhs=tri constant) in the same
    accumulation group - exp then produces exact zeros in the masked region
    and the old DVE trim multiply (a cross-engine hop on the critical path)
    disappears;
  - scores for slot i+1 are emitted before ctx for slot i (per-head
    [128,S] score tiles, double-buffered PSUM);
  - conv matmuls of batch b+1 and out-projection matmuls of batch b-3 are
    chopped into small chunks and drained as PE filler inside the attention
    slots, with the LN/bn/h/transpose chain of b+1 emitted per-s-tile along
    the way.

Per-batch dataflow (as in the baseline): conv -> LN -> hT via 16 xbar
transposes -> fp8 cast (Pool) -> projections (v with a ones column per head
so the ctx matmul also yields the softmax denominator row) -> per-head-pair
attention -> denominator gather (Pool SWDGE) -> reciprocal (DVE) -> [1->64]
HWDGE broadcasts -> one big normalize multiply -> output projection ->
one [128, 4, 512] DRAM store. zero_pad (first query row zeroed) comes from a
column-0 -1e9 variant of the triangle for ki=0 plus a +1e-10 denominator
guard.
"""

import sys

sys.path.insert(0, "/opt/trn_rl_repo")

import numpy as np
import ml_dtypes
from contextlib import ExitStack

import concourse.bass as bass
from concourse import bacc
import concourse.mybir as mybir
import concourse.tile as tile
from concourse.bass_utils import run_bass_kernel_spmd

# Force Exp and Ln to resolve to the single table set that contains both
# (natural_log_exp_and_others), so ACT doesn't thrash table loads.
import concourse.hw_specs as _hw_specs

_orig_get_tables = _hw_specs.get_activation_tables


def _patched_get_tables(arch):
    t = dict(_orig_get_tables(arch))
    exp = mybir.ActivationFunctionType.Exp
    ln = mybir.ActivationFunctionType.Ln
    for name, funcs in t.items():
        if name != "natural_log_exp_and_others" and (exp in funcs or ln in funcs):
            t[name] = funcs - {exp, ln}
    return t


_hw_specs.get_activation_tables = _patched_get_tables
bacc.get_activation_tables = _patched_get_tables

B, S, D, H, KW = 64, 512, 512, 8, 3
DK = D // H  # 64
NCORES = 8
BL = B // NCORES  # 8 batches per core
P = 128
NST = S // P  # 4 s-tiles
NIC = D // P  # 4 input-chunks
NG = 2  # channel pair-groups for DoubleRow (256 ch each)
SP = S + 16  # padded x free dim (2 zero cols + 512 + pad; DR pair stride must be 16B-aligned)
EPS = 1e-12
F32 = mybir.dt.float32
BF16 = mybir.dt.bfloat16
FP8 = mybir.dt.float8e4
AF = mybir.ActivationFunctionType
DR = mybir.MatmulPerfMode.DoubleRow
EXP_SCALE = 0.125 / 4096.0  # 1/8 score scale folded with the x64*x64 qk weight scale
NEG = -1.0e9


def build_nc():
    nc = bacc.Bacc("TRN2", target_bir_lowering=False)
    # x: [b][p][(hilo*2+g)*NG*SP + j*SP + s+2] = fp8 part of x[b, s, g*256+j*128+p]
    xt = nc.declare_dram_parameter("xt", [BL, P, 4 * NG * SP], FP8, isOutput=False)
    # wconv: [hilo*2+g][p][j][k][d] = fp8 part of 16*W'[d, g*256+j*128+p, k]
    wconv = nc.declare_dram_parameter("wconv", [4, P, NG, KW, D], FP8, isOutput=False)
    # wq/wk: [g][p][j][d] = fp8(64*Wq[d, g*256+j*128+p])
    wq = nc.declare_dram_parameter("wq", [NG, P, NG, D], FP8, isOutput=False)
    wk = nc.declare_dram_parameter("wk", [NG, P, NG, D], FP8, isOutput=False)
    wv = nc.declare_dram_parameter("wv", [NIC, P, D], BF16, isOutput=False)
    wo = nc.declare_dram_parameter("wo", [NIC, P, D], BF16, isOutput=False)
    # trin[:, 0, :]: -1e9 strictly-below-diagonal triangle; [:, 1, :]: same + col 0
    trin = nc.declare_dram_parameter("trin", [P, 2, P], BF16, isOutput=False)
    ident = nc.declare_dram_parameter("ident", [P, P], BF16, isOutput=False)
    out = nc.declare_dram_parameter("out", [BL, P, NST, D], F32, isOutput=True)

    with ExitStack() as ctx:
        tc = ctx.enter_context(tile.TileContext(nc))
        singles = ctx.enter_context(tc.tile_pool(name="singles", bufs=1))
        xt_pool = ctx.enter_context(tc.tile_pool(name="xt", bufs=2))
        a_pool = ctx.enter_context(tc.tile_pool(name="a", bufs=5))
        stat_pool = ctx.enter_context(tc.tile_pool(name="stat", bufs=4))
        h_pool = ctx.enter_context(tc.tile_pool(name="h", bufs=4))
        ht_pool = ctx.enter_context(tc.tile_pool(name="ht", bufs=2))
        hh_pool = ctx.enter_context(tc.tile_pool(name="hh", bufs=2))
        qk_pool = ctx.enter_context(tc.tile_pool(name="qk", bufs=12))
        v_pool = ctx.enter_context(tc.tile_pool(name="v", bufs=8))
        pt_pool = ctx.enter_context(tc.tile_pool(name="pt", bufs=4))
        dn_pool = ctx.enter_context(tc.tile_pool(name="dn", bufs=2))
        r_pool = ctx.enter_context(tc.tile_pool(name="r", bufs=3))
        cx_pool = ctx.enter_context(tc.tile_pool(name="cx", bufs=5))
        o_pool = ctx.enter_context(tc.tile_pool(name="o", bufs=2))
        ps_a = ctx.enter_context(tc.tile_pool(name="ps_a", bufs=2, space="PSUM"))
        ps_mm = ctx.enter_context(tc.tile_pool(name="ps_mm", bufs=2, space="PSUM"))
        ps_sc = ctx.enter_context(tc.tile_pool(name="ps_sc", bufs=2, space="PSUM"))
        ps_cx = ctx.enter_context(tc.tile_pool(name="ps_cx", bufs=2, space="PSUM"))

        # --- load weights once ---
        wconv_sb = [singles.tile([P, NG, KW, D], FP8, name=f"wconv{t}", tag=f"wconv{t}") for t in range(4)]
        wq_sb = [singles.tile([P, NG, D], FP8, name=f"wq{g}", tag=f"wq{g}") for g in range(NG)]
        wk_sb = [singles.tile([P, NG, D], FP8, name=f"wk{g}", tag=f"wk{g}") for g in range(NG)]
        wv_sb = [singles.tile([P, D], BF16, name=f"wv{i}", tag=f"wv{i}") for i in range(NIC)]
        wo_sb = [singles.tile([P, D], BF16, name=f"wo{i}", tag=f"wo{i}") for i in range(NIC)]
        trin_sb = singles.tile([P, 2, P], BF16, name="trin", tag="trin")
        ident_sb = singles.tile([P, P], BF16, name="ident", tag="ident")
        eps_sb = singles.tile([P, 1], F32, name="eps", tag="eps")
        nc.vector.memset(eps_sb, EPS)
        tiny_sb = singles.tile([P, 1], F32, name="tiny", tag="tiny")
        nc.vector.memset(tiny_sb, 1e-10)
        zero_sb = singles.tile([P, 1], F32, name="zero", tag="zero")
        nc.vector.memset(zero_sb, 0.0)
        for t in range(4):
            nc.gpsimd.dma_start(out=wconv_sb[t], in_=wconv[t])
        nc.gpsimd.dma_start(out=trin_sb, in_=trin[:])
        nc.gpsimd.dma_start(out=ident_sb, in_=ident[:])
        for g in range(NG):
            nc.gpsimd.dma_start(out=wq_sb[g], in_=wq[g])
            nc.gpsimd.dma_start(out=wk_sb[g], in_=wk[g])
        for i in range(NIC):
            nc.gpsimd.dma_start(out=wv_sb[i], in_=wv[i])
        for i in range(NIC):
            nc.gpsimd.dma_start(out=wo_sb[i], in_=wo[i])

        def load_xt(b):
            xsb = xt_pool.tile([P, 4, NG, SP], FP8, name="xsb", tag="xsb")
            nc.sync.dma_start(out=xsb.rearrange("p a j s -> p (a j s)"), in_=xt[b])
            return xsb

        CONV_TERMS = [(0, 0), (1, 0), (0, 1)]  # (x part, w part): hh, lh, hl
        CHUNK = 6  # conv matmuls per filler chunk

        def make_front_filler(b, xsb, result):
            """Filler closures for conv+LN+h+transposes of batch b.

            result: dict that will receive 'ht' and is later finished with the
            hh cast by finish_front."""
            ht_sb = [ht_pool.tile([P, NG, S], BF16, name=f"ht{g}", tag=f"ht{g}") for g in range(NG)]
            hh_sb = [hh_pool.tile([P, NG, S], FP8, name=f"hh{g}", tag=f"hh{g}") for g in range(NG)]
            mv = stat_pool.tile([P, NST, 2], F32, name="mv", tag="mv")
            result["ht"] = ht_sb
            result["hh"] = hh_sb
            chunks = []
            state = {}

            def conv_chunk(st, lo, hi):
                def emit():
                    if lo == 0:
                        state[st] = ps_a.tile([P, D], F32, name="aps", tag="aps")
                    aps = state[st]
                    for n in range(lo, hi):
                        ds, rem = divmod(n, 18)
                        ti, rem2 = divmod(rem, NG * KW)
                        g, k = divmod(rem2, KW)
                        xi, wi = CONV_TERMS[ti]
                        nc.tensor.matmul(
                            aps[:, ds * 256 : (ds + 1) * 256],
                            lhsT=xsb[:, xi * NG + g, :, st * P + k : st * P + k + P],
                            rhs=wconv_sb[wi * NG + g][:, :, k, ds * 256 : (ds + 1) * 256],
                            start=(rem == 0),
                            stop=(rem == 17),
                            perf_mode=DR,
                        )
                    if hi == 36:
                        # LN + h + transposes for this s-tile
                        asb = a_pool.tile([P, D], F32, name="asb", tag="asb")
                        nc.vector.tensor_copy(asb, aps)
                        stats = stat_pool.tile([P, 6], F32, name="bnst", tag="bnst")
                        nc.vector.bn_stats(out=stats, in_=asb)
                        nc.vector.bn_aggr(out=mv[:, st, :], in_=stats)
                        lnv = stat_pool.tile([P, 1], F32, name="lnv", tag="lnv")
                        rstd = stat_pool.tile([P, 1], F32, name="rstd", tag="rstd")
                        nc.scalar.activation(lnv, mv[:, st, 1:2], AF.Ln, bias=eps_sb, scale=1.0)
                        nc.scalar.activation(rstd, lnv, AF.Exp, bias=zero_sb, scale=-0.5)
                        hsb = h_pool.tile([P, D], BF16, name="hsb", tag="hsb")
                        nc.vector.tensor_scalar(
                            hsb,
                            asb,
                            scalar1=mv[:, st, 0:1],
                            scalar2=rstd,
                            op0=mybir.AluOpType.subtract,
                            op1=mybir.AluOpType.mult,
                        )
                        for i in range(NIC):
                            nc.sync.dma_start(
                                out=ht_sb[i // 2][:, i % 2, st * P : (st + 1) * P],
                                in_=hsb[:, i * P : (i + 1) * P],
                                transpose=True,
                            )

                return emit

            for st in range(NST):
                for c in range(0, 36, CHUNK):
                    chunks.append(conv_chunk(st, c, min(c + CHUNK, 36)))
            return chunks

        def make_outproj_filler(b, csbn):
            osb = o_pool.tile([P, NST, D], F32, name="osb", tag="osb")
            chunks = []

            def op_chunk(st):
                def emit():
                    ops = ps_mm.tile([P, D], F32, name="qps", tag="qps")
                    for hp in range(H // 2):
                        nc.tensor.matmul(
                            ops,
                            lhsT=csbn[:, hp, st * P : (st + 1) * P],
                            rhs=wo_sb[hp],
                            start=(hp == 0),
                            stop=(hp == H // 2 - 1),
                        )
                    nc.scalar.copy(osb[:, st, :], ops)
                    if st == NST - 1:
                        nc.sync.dma_start(out=out[b], in_=osb)

                return emit

            for st in range(NST):
                chunks.append(op_chunk(st))
            return chunks

        def emit_qkv(b, ht_sb, hh_sb):
            v_aug = []
            for st in range(NST):
                vps = ps_mm.tile([P, D], F32, name="qps", tag="qps")
                for i in range(NIC):
                    nc.tensor.matmul(
                        vps,
                        lhsT=ht_sb[i // 2][:, i % 2, st * P : (st + 1) * P],
                        rhs=wv_sb[i],
                        start=(i == 0),
                        stop=(i == NIC - 1),
                    )
                vsb = v_pool.tile([P, H, 66], BF16, name="vsb", tag="vsb")
                nc.vector.memset(vsb[:, :, 64:66], 1.0)
                nc.vector.tensor_copy(
                    vsb[:, :, 0:64], vps.rearrange("p (h d) -> p h d", h=H)
                )
                v_aug.append(vsb)

            qt_sb = []
            kt_sb = []
            for oc in range(NIC):
                for dst, w_sb in ((qt_sb, wq_sb), (kt_sb, wk_sb)):
                    qps = ps_mm.tile([P, S], F32, name="qps", tag="qps")
                    for ss in range(2):
                        for g in range(NG):
                            nc.tensor.matmul(
                                qps[:, ss * 256 : (ss + 1) * 256],
                                lhsT=w_sb[g][:, :, oc * P : (oc + 1) * P],
                                rhs=hh_sb[g][:, :, ss * 256 : ss * 256 + 256],
                                start=(g == 0),
                                stop=(g == NG - 1),
                                perf_mode=DR,
                            )
                    qsb = qk_pool.tile([P, S], BF16, name="qtsb", tag="qtsb")
                    nc.vector.tensor_copy(qsb, qps)
                    dst.append(qsb)
            return qt_sb, kt_sb, v_aug

        def emit_attention(b, qt_sb, kt_sb, v_aug, filler):
            """Per-(pair, ki, head) attention with scores one slot ahead of ctx
            and filler chunks drained into the gaps."""
            dtmp = dn_pool.tile([65, H, S], BF16, name="dtmp", tag="dtmp", bufs=1)
            csb = cx_pool.tile([P, H // 2, S], BF16, name="csb", tag="csb")
            codd = cx_pool.tile([DK, H // 2, S], BF16, name="codd", tag="codd", bufs=2)

            def draw():
                if filler:
                    filler.pop(0)()

            for hp in range(H // 2):
                cps2 = [
                    ps_cx.tile([65, S], F32, name="cps", tag="cps") for _ in range(2)
                ]
                pend = None  # (ki, e, pt)
                for ki in range(NST):
                    qoff = ki * P
                    nq = S - qoff
                    for e in range(2):
                        hr = e * DK
                        sps = ps_sc.tile([P, S], F32, name="sps", tag="sps")
                        # scores first (start=True zeroes the full [0:nq]),
                        # then the -1e9 causal triangle accumulates onto the
                        # 128-wide diagonal block.
                        nc.tensor.matmul(
                            sps[:, 0:nq],
                            lhsT=kt_sb[hp][hr : hr + DK, ki * P : (ki + 1) * P],
                            rhs=qt_sb[hp][hr : hr + DK, qoff:S],
                            start=True,
                            stop=False,
                        )
                        nc.tensor.matmul(
                            sps[:, 0:P],
                            lhsT=ident_sb,
                            rhs=trin_sb[:, 1 if ki == 0 else 0, :],
                            start=False,
                            stop=True,
                        )
                        pt = pt_pool.tile([P, S], BF16, name="pt", tag="pt")
                        nc.scalar.activation(pt[:, 0:nq], sps[:, 0:nq], AF.Exp, scale=EXP_SCALE)
                        if pend is not None:
                            pki, pe, ppt = pend
                            pnq = S - pki * P
                            nc.tensor.matmul(
                                cps2[pe][:, pki * P : S],
                                lhsT=v_aug[pki][:, 2 * hp + pe, 0:65],
                                rhs=ppt[:, 0:pnq],
                                start=(pki == 0),
                                stop=(pki == NST - 1),
                            )
                        draw()
                        pend = (ki, e, pt)
                pki, pe, ppt = pend
                nc.tensor.matmul(
                    cps2[pe][:, pki * P : S],
                    lhsT=v_aug[pki][:, 2 * hp + pe, 0:65],
                    rhs=ppt[:, 0 : S - pki * P],
                    start=(pki == 0),
                    stop=(pki == NST - 1),
                )
                # denominator rows -> staging (row 64), +tiny guard for q=0
                nc.scalar.activation(
                    dtmp[64:65, 2 * hp, :], cps2[0][64:65, :], AF.Identity, bias=tiny_sb[64:65, :], scale=1.0
                )
                nc.scalar.activation(
                    dtmp[64:65, 2 * hp + 1, :], cps2[1][64:65, :], AF.Identity, bias=tiny_sb[64:65, :], scale=1.0
                )
                # unnormalized ctx: even head -> rows 0-63 direct; odd head
                # staged for one partition-shifting DMA at the end.
                nc.scalar.copy(csb[0:DK, hp, :], cps2[0][0:DK, :])
                nc.scalar.copy(codd[:, hp, :], cps2[1][0:DK, :])

            nc.sync.dma_start(out=csb[DK:P, :, :], in_=codd)
            dcat = dn_pool.tile([H, S], F32, name="dcat", tag="dcat")
            nc.gpsimd.dma_start(out=dcat, in_=dtmp[64:65, :, :])
            return (b, csb, dcat)

        def denom_chain(b, dcat):
            rcat = dn_pool.tile([H, S], F32, name="rcat", tag="rcat")
            nc.vector.reciprocal_approx_fast(out=rcat, in_=dcat)
            rcb = dn_pool.tile([H, S], BF16, name="rcb", tag="rcb")
            nc.vector.tensor_copy(rcb, rcat)
            rec = r_pool.tile([P, H // 2, S], BF16, name="rec", tag="rec")
            for hp in range(H // 2):
                # one DMA per pair: src iterates (row in {2hp, 2hp+1}, dup 64, s)
                # which matches dest partitions p = 64*row + dup exactly.
                rsrc = rcb[2 * hp : 2 * hp + 2, :]
                rsrc = bass.AP(
                    tensor=rsrc.tensor,
                    offset=rsrc.offset,
                    ap=[rsrc.ap[0], [0, DK], [1, S]],
                )
                nc.sync.dma_start(out=rec[:, hp, :], in_=rsrc)
            return rec

        def tail_norm(b, csb, rec):
            csbn = cx_pool.tile([P, H // 2, S], BF16, name="csbn", tag="csbn", bufs=3)
            nc.vector.tensor_mul(csbn, csb, rec)
            return (b, csbn)

        # ---------------- pipeline ----------------
        # state per stage, keyed by batch:
        #   front(b):   conv filler emitted during iteration b; ht/hh ready end of b
        #   mid(b):     qkv + attention emitted in iteration b+1
        #   denom(b):   reciprocal+broadcasts in iteration b+2
        #   norm(b):    csbn multiply in iteration b+3
        #   outproj(b): filler during iteration b+4
        front_res = {}  # b -> {"ht":..., "hh":...}
        mid_res = {}  # b -> (b, csb, dcat)
        rec_res = {}  # b -> rec
        norm_res = {}  # b -> (b, csbn)

        xsb_cur = load_xt(0)
        for b in range(BL + 4):
            filler = []
            if b < BL:
                if b + 1 < BL:
                    xsb_next = load_xt(b + 1)
                front_res[b] = {}
                filler += make_front_filler(b, xsb_cur, front_res[b])
                if b + 1 < BL:
                    xsb_cur = xsb_next
            if b - 3 >= 0 and (b - 3) in norm_res:
                nb, csbn = norm_res.pop(b - 3)
                filler += make_outproj_filler(nb, csbn)
            if b - 2 >= 0 and (b - 2) in rec_res:
                mb, csb_, dcat_ = mid_res.pop(b - 2)
                norm_res[b - 2] = tail_norm(mb, csb_, rec_res.pop(b - 2))
            if b - 1 >= 0 and (b - 1) in front_res:
                # pre-draw a few ready conv chunks so the PE has work while
                # the hh cast of b-1 settles
                for _ in range(min(3, len(filler))):
                    filler.pop(0)()
                fr = front_res.pop(b - 1)
                qt, kt, va = emit_qkv(b - 1, fr["ht"], fr["hh"])
                mid_res[b - 1] = emit_attention(b - 1, qt, kt, va, filler)
            # drain leftover filler (startup/drain iterations)
            for f in filler:
                f()
            if b < BL:
                fr = front_res[b]
                for g in range(NG):
                    nc.gpsimd.tensor_copy(fr["hh"][g], fr["ht"][g])
            if b - 1 >= 0 and (b - 1) in mid_res:
                _, _, dcat_ = mid_res[b - 1]
                rec_res[b - 1] = denom_chain(b - 1, dcat_)

    nc.compile()
    return nc


def prep_inputs(inputs):
    """Host-side prep: shard over batch, fold scales into weights, fp8 splits."""
    x = np.asarray(inputs["x"], np.float32)
    conv_w = np.asarray(inputs["conv_w"], np.float32)
    conv_b = np.asarray(inputs["conv_b"], np.float32)
    sb = np.asarray(inputs["sqrt_beta"], np.float32).reshape(D)
    ln_w = np.asarray(inputs["ln_w"], np.float32)
    ln_b = np.asarray(inputs["ln_b"], np.float32)
    Wq = np.asarray(inputs["Wq"], np.float32)
    Wk = np.asarray(inputs["Wk"], np.float32)
    Wv = np.asarray(inputs["Wv"], np.float32)
    Wo = np.asarray(inputs["Wo"], np.float32)
    mask = np.asarray(inputs["mask"])

    for nm in ("bq", "bk", "bv", "bo"):
        assert not np.any(np.asarray(inputs[nm])), f"{nm} must be zero"
    assert not np.any(conv_b), "conv_b must be zero"
    assert not np.any(ln_b), "ln_b must be zero"
    assert np.array_equal(
        mask.reshape(S, S), np.tril(np.ones((S, S), mask.dtype))
    ), "mask must be causal"

    bf = ml_dtypes.bfloat16
    f8 = ml_dtypes.float8_e4m3fn

    c1 = 1.0 - sb * sb
    c2 = 1.0 + sb * sb
    Wp = conv_w * c1[:, None, None]  # [o, i, k]
    Wp[np.arange(D), np.arange(D), 2] += c2
    Wp16 = Wp * 16.0  # x16: diag taps (up to ~17) must stay under e4m3 max 448
    Wph = Wp16.astype(f8)
    Wpl = (Wp16 - Wph.astype(np.float32)).astype(f8)
    # wconv[hilo*2+g][p][j][k][d] = part[d, g*256+j*128+p, k]
    wconv = np.empty((4, P, NG, KW, D), f8)
    for t, Wpart in enumerate((Wph, Wpl)):
        r = np.ascontiguousarray(Wpart.transpose(1, 2, 0)).reshape(NG, NG, P, KW, D)
        wconv[2 * t : 2 * t + 2] = r.transpose(0, 2, 1, 3, 4)

    def fold_qk(W):  # [o, i] -> [g, p, j, o], fp8(64*W*ln_w)
        Wf = (64.0 * W * ln_w[None, :]).astype(f8)
        r = np.ascontiguousarray(Wf.T).reshape(NG, NG, P, D)
        return r.transpose(0, 2, 1, 3).copy()

    wq_h, wk_h = fold_qk(Wq), fold_qk(Wk)

    def fold(W):  # [o, i] -> [ic, il, o] with ln_w folded on i
        Wf = W * ln_w[None, :]
        return np.ascontiguousarray(Wf.T).reshape(NIC, P, D)

    wv_h = fold(Wv).astype(bf)
    wo_h = np.ascontiguousarray(Wo.T).reshape(NIC, P, D).astype(bf)

    # trin[k, v, q]: v=0 plain causal (-1e9 where q < k), v=1 also col 0 = -1e9
    tri = np.where(np.arange(P)[None, :] < np.arange(P)[:, None], NEG, 0.0).astype(np.float32)
    tri0 = tri.copy()
    tri0[:, 0] = NEG
    trin = np.stack([tri, tri0], axis=1)  # [P, 2, P]

    consts = {
        "wconv": wconv,
        "wq": wq_h,
        "wk": wk_h,
        "wv": wv_h,
        "wo": wo_h,
        "trin": trin.astype(bf),
        "ident": np.eye(P, dtype=bf),
    }

    in_maps = []
    for c in range(NCORES):
        xs = x[c * BL : (c + 1) * BL]  # [BL, S, D]
        xh = xs.astype(f8)
        xl = (xs - xh.astype(np.float32)).astype(f8)
        xtp = np.zeros((BL, P, 4, NG, SP), f8)
        for t, xpart in enumerate((xh, xl)):
            r = np.ascontiguousarray(xpart.transpose(0, 2, 1)).reshape(BL, NG, NG, P, S)
            xtp[:, :, 2 * t : 2 * t + 2, :, 2 : 2 + S] = r.transpose(0, 3, 1, 2, 4)
        m = dict(consts)
        m["xt"] = np.ascontiguousarray(xtp).reshape(BL, P, 4 * NG * SP)
        in_maps.append(m)
    return in_maps


_NC_CACHE = {}


def get_nc():
    if "nc" not in _NC_CACHE:
        _NC_CACHE["nc"] = build_nc()
    return _NC_CACHE["nc"]


def unpack_out(arr):
    # [BL, P, NST, D] -> [BL, S, D] (s = st*P + p)
    a = np.asarray(arr, np.float32).reshape(BL, P, NST, D)
    return np.ascontiguousarray(a.transpose(0, 2, 1, 3)).reshape(BL, S, D)


def kernel(**inputs):
    nc = get_nc()
    in_maps = prep_inputs(inputs)
    res = run_bass_kernel_spmd(nc, in_maps, list(range(NCORES)))
    return np.concatenate([unpack_out(r["out"]) for r in res.results], axis=0)


if __name__ == "__main__":
    nc = build_nc()
    print("built ok")
